# revision 1
# baseline (speedup 1.0000x reference)
"""GCNConv (batched dense-adjacency GraphConv) Trainium2 kernel.

Math: out[b] = sum_n relu((A[b] @ X[b]) @ W1 + b1) @ W2 + N * b2
Using (A X) W1 == A (X W1), precompute Y = X @ W1 on host (tiny), so the
device only does the memory-bound part: Z^T = Y^T A^T per batch, then
relu+bias+column-sum on the Activation engine. Host finishes with the
[B,4] @ [4,1] readout.

Sharding: pure data parallel over the batch dim, 32 batches per core x 8.
"""

import sys

if "/opt/trn_rl_repo" not in sys.path:
    sys.path.insert(0, "/opt/trn_rl_repo")

import numpy as np

import concourse.bass as bass  # noqa: F401  (import keeps bass registered)
import concourse.mybir as mybir
import concourse.tile as tile
from concourse import bacc
from concourse.bass_utils import run_bass_kernel_spmd

N_CORES = 8
B, N, F = 256, 512, 2
H = 4  # hidden dim after W1
BPC = B // N_CORES  # batches per core
NCH = N // 128  # m-chunks per batch

# "fp32": exact fp32 matmuls (4 cyc/row on PE)
# "f32r": fp32 storage, relaxed-precision full-rate matmul
# "bf16x2": hi/lo bf16 split (fp32-like accuracy, full-rate PE, same DMA bytes)
# "bf16": single bf16 (half DMA bytes, ~1e-3 rel err)
STRATEGY = "fp32"

_BUILT = {}


def _build(strategy):
    """Build + compile the Bass module (once per process per strategy)."""
    f32 = mybir.dt.float32
    if strategy in ("fp32", "f32r"):
        in_dt = mybir.dt.float32r if strategy == "f32r" else f32
        n_pass = 1
    elif strategy in ("bf16x2", "bf16"):
        in_dt = mybir.dt.bfloat16
        n_pass = 2 if strategy == "bf16x2" else 1
    else:
        raise ValueError(strategy)

    nc = bacc.Bacc("TRN2", target_bir_lowering=False, debug=False,
                   num_devices=N_CORES)

    # a[b] is A[b]^T packed as [128, NCH*512]: a[b][p][c*512+n] = A[b][n][c*128+p]
    ats = [nc.dram_tensor(f"at{i}", [BPC, 128, NCH * N], in_dt,
                          kind="ExternalInput") for i in range(n_pass)]
    # y packed [128, BPC*NCH*H]: y[p][(b*NCH+c)*H+j] = Y[b][c*128+p][j]
    ys = [nc.dram_tensor(f"y{i}", [128, BPC * NCH * H], in_dt,
                         kind="ExternalInput") for i in range(n_pass)]
    b1d = nc.dram_tensor("b1", [H, 1], f32, kind="ExternalInput")
    outd = nc.dram_tensor("out", [H, BPC], f32, kind="ExternalOutput")

    with tile.TileContext(nc) as tc:
        with tc.tile_pool(name="const", bufs=1) as constp, \
             tc.tile_pool(name="apool", bufs=4 if strategy != "bf16" else 6) as apool, \
             tc.tile_pool(name="scratch", bufs=2) as spool, \
             tc.tile_pool(name="psum", bufs=2, space="PSUM") as ppool:
            b1_t = constp.tile([H, 1], f32)
            nc.sync.dma_start(out=b1_t[:], in_=b1d[:])
            y_ts = []
            for i in range(n_pass):
                y_t = constp.tile([128, BPC * NCH * H], in_dt, tag=f"y{i}")
                nc.sync.dma_start(out=y_t[:], in_=ys[i][:])
                y_ts.append(y_t)
            out_t = constp.tile([H, BPC], f32)

            for b in range(BPC):
                a_ts = []
                for i in range(n_pass):
                    a_t = apool.tile([128, NCH * N], in_dt, tag=f"a{i}")
                    nc.sync.dma_start(out=a_t[:], in_=ats[i][b])
                    a_ts.append(a_t)
                ps = ppool.tile([H, N], f32)
                # accumulation passes: hi@hi, then (for bf16x2) lo@hi + hi@lo
                passes = [(0, 0)] if n_pass == 1 else [(0, 0), (1, 0), (0, 1)]
                nmm = len(passes) * NCH
                k = 0
                for (yi, ai) in passes:
                    for c in range(NCH):
                        nc.tensor.matmul(
                            ps[:],
                            y_ts[yi][:, (b * NCH + c) * H:(b * NCH + c + 1) * H],
                            a_ts[ai][:, c * N:(c + 1) * N],
                            start=(k == 0), stop=(k == nmm - 1),
                        )
                        k += 1
                sc = spool.tile([H, N], f32)
                nc.scalar.activation(
                    sc[:], ps[:], mybir.ActivationFunctionType.Relu,
                    bias=b1_t[:], scale=1.0,
                    accum_out=out_t[:, b:b + 1],
                )
            nc.sync.dma_start(out=outd[:], in_=out_t[:])

    nc.compile()
    return nc


def _get_nc(strategy=None):
    strategy = strategy or STRATEGY
    if strategy not in _BUILT:
        _BUILT[strategy] = _build(strategy)
    return _BUILT[strategy]


def _pack_at(adj):
    """[Bc, N, N] f32 -> A^T packed [Bc, 128, NCH*N] (see _build)."""
    # at_packed[b, p, c*N + n] = adj[b, n, c*128 + p]
    t = adj.reshape(adj.shape[0], N, NCH, 128)  # [b, n, c, p]
    return np.ascontiguousarray(t.transpose(0, 3, 2, 1)).reshape(
        adj.shape[0], 128, NCH * N)


def _pack_y(y):
    """[Bc, N, H] f32 -> [128, Bc*NCH*H] (see _build)."""
    bc = y.shape[0]
    t = y.reshape(bc, NCH, 128, H)  # [b, c, p, j]
    return np.ascontiguousarray(t.transpose(2, 0, 1, 3)).reshape(128, bc * NCH * H)


def _prep_in_maps(node_features, adj_matrices, W1, b1, strategy):
    import ml_dtypes
    y_full = np.einsum("bnf,fh->bnh", node_features, W1).astype(np.float32)
    b1_col = np.asarray(b1, np.float32).reshape(H, 1)
    in_maps = []
    for c in range(N_CORES):
        sl = slice(c * BPC, (c + 1) * BPC)
        at = _pack_at(np.ascontiguousarray(adj_matrices[sl]))
        yp = _pack_y(y_full[sl])
        m = {"b1": b1_col}
        if strategy in ("fp32", "f32r"):
            m["at0"], m["y0"] = at, yp
        elif strategy == "bf16":
            m["at0"] = at.astype(ml_dtypes.bfloat16)
            m["y0"] = yp.astype(ml_dtypes.bfloat16)
        else:  # bf16x2
            at_hi = at.astype(ml_dtypes.bfloat16)
            y_hi = yp.astype(ml_dtypes.bfloat16)
            m["at0"], m["y0"] = at_hi, y_hi
            m["at1"] = (at - at_hi.astype(np.float32)).astype(ml_dtypes.bfloat16)
            m["y1"] = (yp - y_hi.astype(np.float32)).astype(ml_dtypes.bfloat16)
        in_maps.append(m)
    return in_maps


def _finish(results, W2, b2):
    # results[c]["out"]: [H, BPC]; colsum[b, j] = sum_n relu(Z + b1)
    cols = np.stack([r["out"] for r in results])  # [8, H, BPC]
    colsum = cols.transpose(0, 2, 1).reshape(B, H).astype(np.float32)
    out = colsum @ np.asarray(W2, np.float32) + N * np.asarray(b2, np.float32)
    return out.astype(np.float32)


def kernel(node_features, adj_matrices, W1, b1, W2, b2):
    node_features = np.asarray(node_features, np.float32)
    adj_matrices = np.asarray(adj_matrices, np.float32)
    nc = _get_nc()
    in_maps = _prep_in_maps(node_features, adj_matrices, W1, b1, STRATEGY)
    res = run_bass_kernel_spmd(nc, in_maps, core_ids=list(range(N_CORES)))
    return _finish(res.results, W2, b2)


# revision 4
# speedup vs baseline: 115289.6001x; 115289.6001x over previous
"""GCNConv (batched dense-adjacency GraphConv) Trainium2 kernel.

Math: out[b] = sum_n relu((A[b] @ X[b]) @ W1 + b1) @ W2 + N * b2
Using (A X) W1 == A (X W1), precompute Y = X @ W1 on host (tiny), so the
device only does the memory-bound part: Z^T = Y^T A^T per batch, then
relu+bias+column-sum on the Activation engine. Host finishes with the
[B,4] @ [4,1] readout.

Sharding: pure data parallel over the batch dim, 32 batches per core x 8.
"""

import sys

if "/opt/trn_rl_repo" not in sys.path:
    sys.path.insert(0, "/opt/trn_rl_repo")

import numpy as np

import concourse.bass as bass  # noqa: F401  (import keeps bass registered)
import concourse.mybir as mybir
import concourse.tile as tile
from concourse import bacc
from concourse.bass_utils import run_bass_kernel_spmd

N_CORES = 8
B, N, F = 256, 512, 2
H = 4  # hidden dim after W1
BPC = B // N_CORES  # batches per core
NCH = N // 128  # m-chunks per batch

# "fp32": exact fp32 matmuls (4 cyc/row on PE)
# "f32r": fp32 storage, relaxed-precision full-rate matmul
# "bf16x2": hi/lo bf16 split (fp32-like accuracy, full-rate PE, same DMA bytes)
# "bf16": single bf16 (half DMA bytes, ~1e-3 rel err)
STRATEGY = "fp32"

_BUILT = {}


def _build(strategy, repeat=1):
    """Build + compile the Bass module (once per process per strategy).

    repeat > 1 wraps the per-batch loop in a device-side For loop that
    re-runs the whole workload `repeat` times — used only for timing
    (amortizes host dispatch overhead over many on-device iterations).
    """
    f32 = mybir.dt.float32
    if strategy in ("fp32", "f32r"):
        in_dt = mybir.dt.float32r if strategy == "f32r" else f32
        n_pass = 1
    elif strategy in ("bf16x2", "bf16"):
        in_dt = mybir.dt.bfloat16
        n_pass = 2 if strategy == "bf16x2" else 1
    else:
        raise ValueError(strategy)

    nc = bacc.Bacc("TRN2", target_bir_lowering=False, debug=False,
                   num_devices=N_CORES)

    # a[b] is A[b]^T packed as [128, NCH*512]: a[b][p][c*512+n] = A[b][n][c*128+p]
    ats = [nc.dram_tensor(f"at{i}", [BPC, 128, NCH * N], in_dt,
                          kind="ExternalInput") for i in range(n_pass)]
    # y packed [128, BPC*NCH*H]: y[p][(b*NCH+c)*H+j] = Y[b][c*128+p][j]
    ys = [nc.dram_tensor(f"y{i}", [128, BPC * NCH * H], in_dt,
                         kind="ExternalInput") for i in range(n_pass)]
    b1d = nc.dram_tensor("b1", [H, 1], f32, kind="ExternalInput")
    outd = nc.dram_tensor("out", [H, BPC], f32, kind="ExternalOutput")

    with tile.TileContext(nc) as tc:
        with tc.tile_pool(name="const", bufs=1) as constp, \
             tc.tile_pool(name="apool", bufs=4 if strategy != "bf16" else 6) as apool, \
             tc.tile_pool(name="scratch", bufs=2) as spool, \
             tc.tile_pool(name="psum", bufs=2, space="PSUM") as ppool:
            b1_t = constp.tile([H, 1], f32)
            nc.sync.dma_start(out=b1_t[:], in_=b1d[:])
            y_ts = []
            for i in range(n_pass):
                y_t = constp.tile([128, BPC * NCH * H], in_dt, tag=f"y{i}")
                nc.sync.dma_start(out=y_t[:], in_=ys[i][:])
                y_ts.append(y_t)
            out_t = constp.tile([H, BPC], f32)

            def batch_body(b):
                a_ts = []
                for i in range(n_pass):
                    a_t = apool.tile([128, NCH * N], in_dt, tag=f"a{i}")
                    nc.sync.dma_start(out=a_t[:], in_=ats[i][b])
                    a_ts.append(a_t)
                ps = ppool.tile([H, N], f32)
                # accumulation passes: hi@hi, then (for bf16x2) lo@hi + hi@lo
                passes = [(0, 0)] if n_pass == 1 else [(0, 0), (1, 0), (0, 1)]
                nmm = len(passes) * NCH
                k = 0
                for (yi, ai) in passes:
                    for c in range(NCH):
                        nc.tensor.matmul(
                            ps[:],
                            y_ts[yi][:, (b * NCH + c) * H:(b * NCH + c + 1) * H],
                            a_ts[ai][:, c * N:(c + 1) * N],
                            start=(k == 0), stop=(k == nmm - 1),
                        )
                        k += 1
                sc = spool.tile([H, N], f32)
                nc.scalar.activation(
                    sc[:], ps[:], mybir.ActivationFunctionType.Relu,
                    bias=b1_t[:], scale=1.0,
                    accum_out=out_t[:, b:b + 1],
                )

            if repeat == 1:
                for b in range(BPC):
                    batch_body(b)
            else:
                with tc.For_i(0, repeat, 1):
                    for b in range(BPC):
                        batch_body(b)
            nc.sync.dma_start(out=outd[:], in_=out_t[:])

    nc.compile()
    return nc


def _get_nc(strategy=None, repeat=1):
    strategy = strategy or STRATEGY
    key = (strategy, repeat)
    if key not in _BUILT:
        _BUILT[key] = _build(strategy, repeat)
    return _BUILT[key]


def _pack_at(adj):
    """[Bc, N, N] f32 -> A^T packed [Bc, 128, NCH*N] (see _build)."""
    # at_packed[b, p, c*N + n] = adj[b, n, c*128 + p]
    t = adj.reshape(adj.shape[0], N, NCH, 128)  # [b, n, c, p]
    return np.ascontiguousarray(t.transpose(0, 3, 2, 1)).reshape(
        adj.shape[0], 128, NCH * N)


def _pack_y(y):
    """[Bc, N, H] f32 -> [128, Bc*NCH*H] (see _build)."""
    bc = y.shape[0]
    t = y.reshape(bc, NCH, 128, H)  # [b, c, p, j]
    return np.ascontiguousarray(t.transpose(2, 0, 1, 3)).reshape(128, bc * NCH * H)


def _prep_in_maps(node_features, adj_matrices, W1, b1, strategy):
    import ml_dtypes
    y_full = np.einsum("bnf,fh->bnh", node_features, W1).astype(np.float32)
    b1_col = np.asarray(b1, np.float32).reshape(H, 1)
    in_maps = []
    for c in range(N_CORES):
        sl = slice(c * BPC, (c + 1) * BPC)
        at = _pack_at(np.ascontiguousarray(adj_matrices[sl]))
        yp = _pack_y(y_full[sl])
        m = {"b1": b1_col}
        if strategy in ("fp32", "f32r"):
            m["at0"], m["y0"] = at, yp
        elif strategy == "bf16":
            m["at0"] = at.astype(ml_dtypes.bfloat16)
            m["y0"] = yp.astype(ml_dtypes.bfloat16)
        else:  # bf16x2
            at_hi = at.astype(ml_dtypes.bfloat16)
            y_hi = yp.astype(ml_dtypes.bfloat16)
            m["at0"], m["y0"] = at_hi, y_hi
            m["at1"] = (at - at_hi.astype(np.float32)).astype(ml_dtypes.bfloat16)
            m["y1"] = (yp - y_hi.astype(np.float32)).astype(ml_dtypes.bfloat16)
        in_maps.append(m)
    return in_maps


def _finish(results, W2, b2):
    # results[c]["out"]: [H, BPC]; colsum[b, j] = sum_n relu(Z + b1)
    cols = np.stack([r["out"] for r in results])  # [8, H, BPC]
    colsum = cols.transpose(0, 2, 1).reshape(B, H).astype(np.float32)
    out = colsum @ np.asarray(W2, np.float32) + N * np.asarray(b2, np.float32)
    return out.astype(np.float32)


def kernel(node_features, adj_matrices, W1, b1, W2, b2):
    node_features = np.asarray(node_features, np.float32)
    adj_matrices = np.asarray(adj_matrices, np.float32)
    nc = _get_nc()
    in_maps = _prep_in_maps(node_features, adj_matrices, W1, b1, STRATEGY)
    res = run_bass_kernel_spmd(nc, in_maps, core_ids=list(range(N_CORES)))
    return _finish(res.results, W2, b2)
